# revision 16
# baseline (speedup 1.0000x reference)
"""Equivariant PQ-layer conv kernel for 8x TRN2 NeuronCores.

Strategy: the layer is a 3D conv (SAME, 5^3 taps) with an assembled
(320, 320, 125) kernel over a (320, 8^3) input. The host assembles the
conv kernel from the geometry/weight inputs (cheap: ~0.2 GFLOP vs the
13 GFLOP conv) and shards the 125 taps across the 8 cores (16 tap-slots
per core, 3 slots zero-padded). Each core computes, for its taps,
partial-sum matmuls over all (i_chunk, o_chunk) 128-blocks with the
512 voxels as the free dim, accumulating in PSUM across taps. The host
sums the 8 partials (the unshard for a contraction-parallel split) and
adds the bias.
"""
import os
import numpy as np

C0, C1 = 8, 4
K = 5
G = 8
EPS = 1e-6
R_MAX = 5.5
DIM = C0 + 3 * C1          # 20
Q = 16
P = 8
NCH = DIM * Q              # 320
NV = P * P * P             # 512
K3 = K ** 3                # 125
NCORES = 8
TSLOT = 16                 # tap slots per core: 8*16 = 128 >= 125
CHUNKS = [(0, 128), (128, 128), (256, 64)]

LAST = None                # BassKernelResults of the most recent run (for test harness)
_PROGRAM = None


def _levi_civita():
    e = np.zeros((3, 3, 3), np.float32)
    e[0, 1, 2] = e[1, 2, 0] = e[2, 0, 1] = 1.0
    e[0, 2, 1] = e[2, 1, 0] = e[1, 0, 2] = -1.0
    return e


def _assemble_kern(q_in, q_out, w_ss, w_vs, w_sv, w_vv0, w_vv1):
    """Mirror of the reference kernel assembly, in f32 numpy. -> (320, 320, 125)."""
    offs = np.arange(K, dtype=np.float32) - (K - 1) / 2.0
    oz, oy, ox = np.meshgrid(offs, offs, offs, indexing='ij')
    p_off = np.stack([oz, oy, ox], -1).reshape(-1, 3)
    v = p_off[None, None] - (q_out[:, None, None] - q_in[None, :, None])
    r = np.linalg.norm(v, axis=-1)
    u = np.where(r[..., None] > EPS, v / np.maximum(r, EPS)[..., None], 0.0).astype(np.float32)
    centers = np.linspace(0.0, R_MAX, G).astype(np.float32)
    sigma = R_MAX / (G - 1)
    R = np.exp(-0.5 * ((r[..., None] - centers) / sigma) ** 2).astype(np.float32)
    RY = R[..., None] * u[..., None, :]
    eye3 = np.eye(3, dtype=np.float32)
    eps3 = _levi_civita()
    K_ss = np.einsum('acg,pqkg->apcqk', w_ss, R, optimize=True)
    K_vs = np.einsum('acg,pqkgm->ampcqk', w_vs, RY, optimize=True)
    K_sv = np.einsum('acg,pqkgm->apcmqk', w_sv, RY, optimize=True)
    K_vv = (np.einsum('acg,pqkg,mn->ampcnqk', w_vv0, R, eye3, optimize=True)
            + np.float32(0.7071067811865476) *
            np.einsum('acg,pqkgm,imj->aipcjqk', w_vv1, RY, eps3, optimize=True))
    Qo, Qi = q_out.shape[0], q_in.shape[0]
    top = np.concatenate([K_ss, K_sv.reshape(C0, Qo, 3 * C1, Qi, K3)], axis=2)
    bot = np.concatenate([K_vs.reshape(3 * C1, Qo, C0, Qi, K3),
                          K_vv.reshape(3 * C1, Qo, 3 * C1, Qi, K3)], axis=2)
    kern = np.concatenate([top, bot], axis=0)
    return np.ascontiguousarray(kern.reshape(DIM * Qo, DIM * Qi, K3).astype(np.float32))


def _build_program():
    """One SPMD program: 16 tap-slots of (ker, xsh) -> partial conv output.

    Raw bass (no Tile): explicit engine blocks + standalone wait_ge
    instructions, one DMA-completion semaphore per tap slot.
    """
    global _PROGRAM
    if _PROGRAM is not None:
        return _PROGRAM
    from contextlib import ExitStack
    from concourse import bass, mybir

    nc = bass.Bass("TRN2", target_bir_lowering=False, debug=False,
                   enable_asserts=False, num_devices=NCORES)
    # i-chunks packed side-by-side in the free dim (chunk2 rows 64:128 zeroed)
    ker_d = nc.dram_tensor("ker", [TSLOT, 128, 3 * NCH], mybir.dt.float16,
                           kind="ExternalInput").ap()
    xsh_d = nc.dram_tensor("xsh", [TSLOT, 128, 3 * NV], mybir.dt.float16,
                           kind="ExternalInput").ap()
    # (128, 3*512): o-chunk oc lives in columns [oc*512, (oc+1)*512); host unpacks.
    out_d = nc.dram_tensor("out_part", [128, 3 * NV], mybir.dt.float32,
                           kind="ExternalOutput").ap()

    with ExitStack() as ctx:
        ktiles = [ctx.enter_context(nc.sbuf_tensor(f"kt{t}", [128, 3 * NCH], mybir.dt.float16))
                  for t in range(TSLOT)]
        xtiles = [ctx.enter_context(nc.sbuf_tensor(f"xt{t}", [128, 3 * NV], mybir.dt.float16))
                  for t in range(TSLOT)]
        otile = ctx.enter_context(nc.sbuf_tensor("otile", [128, 3 * NV], mybir.dt.float32))
        psum = [ctx.enter_context(nc.psum_tensor(f"psum{i}", [osz, NV], mybir.dt.float32))
                for i, (o0, osz) in enumerate(CHUNKS)]
        dsem = [ctx.enter_context(nc.semaphore(f"dsem{t}")) for t in range(TSLOT)]
        psem = ctx.enter_context(nc.semaphore("psem"))
        vsem = ctx.enter_context(nc.semaphore("vsem"))
        osem = ctx.enter_context(nc.semaphore("osem"))
        block = ctx.enter_context(nc.Block())

        @block.sync
        def _(sync):
            for t in range(TSLOT):
                sync.dma_start(out=ktiles[t][:, :], in_=ker_d[t, :, :]).then_inc(dsem[t], 16)
                sync.dma_start(out=xtiles[t][:, :], in_=xsh_d[t, :, :]).then_inc(dsem[t], 16)

        @block.tensor
        def _(tensor):
            for t in range(TSLOT):
                tensor.wait_ge(dsem[t], 32)
                for oc, (o0, osz) in enumerate(CHUNKS):
                    for ci, (i0, isz) in enumerate(CHUNKS):
                        mm = tensor.matmul(
                            psum[oc][:, :],
                            ktiles[t][:isz, ci * NCH + o0:ci * NCH + o0 + osz],
                            xtiles[t][:isz, ci * NV:(ci + 1) * NV],
                            start=(t == 0 and ci == 0),
                            stop=(t == TSLOT - 1 and ci == len(CHUNKS) - 1))
                        if t == TSLOT - 1 and oc == len(CHUNKS) - 1 and ci == len(CHUNKS) - 1:
                            mm.then_inc(psem, 1)

        @block.vector
        def _(vector):
            vector.wait_ge(psem, 1)
            for oc, (o0, osz) in enumerate(CHUNKS):
                vector.tensor_copy(
                    otile[:osz, oc * NV:(oc + 1) * NV], psum[oc][:, :]
                ).then_inc(vsem, 1)

        @block.gpsimd
        def _(gpsimd):
            gpsimd.wait_ge(vsem, len(CHUNKS))
            gpsimd.dma_start(out=out_d[:, :], in_=otile[:, :]).then_inc(osem, 16)
            gpsimd.wait_ge(osem, 16)

    _PROGRAM = nc
    return nc


def kernel(x, q_in, q_out, w_ss, w_vs, w_sv, w_vv0, w_vv1, bias):
    global LAST
    from concourse.bass_utils import run_bass_kernel_spmd

    kern = _assemble_kern(np.asarray(q_in, np.float32), np.asarray(q_out, np.float32),
                          np.asarray(w_ss, np.float32), np.asarray(w_vs, np.float32),
                          np.asarray(w_sv, np.float32), np.asarray(w_vv0, np.float32),
                          np.asarray(w_vv1, np.float32))
    xr = np.asarray(x, np.float32).reshape(NCH, P, P, P)
    x_pad = np.zeros((NCH, P + 4, P + 4, P + 4), np.float32)
    x_pad[:, 2:10, 2:10, 2:10] = xr

    xsh = np.empty((K3, NCH, NV), np.float16)
    t = 0
    for dz in range(K):
        for dy in range(K):
            for dx in range(K):
                xsh[t] = x_pad[:, dz:dz + 8, dy:dy + 8, dx:dx + 8].reshape(NCH, NV)
                t += 1
    kerT = np.ascontiguousarray(kern.transpose(2, 1, 0)).astype(np.float16)  # (125, i, o)

    in_maps = []
    for c in range(NCORES):
        taps = list(range(c, K3, NCORES))
        ker_c = np.zeros((TSLOT, 128, 3 * NCH), np.float16)
        xsh_c = np.zeros((TSLOT, 128, 3 * NV), np.float16)
        for s, tp in enumerate(taps):
            for ci, (i0, isz) in enumerate(CHUNKS):
                ker_c[s, :isz, ci * NCH:(ci + 1) * NCH] = kerT[tp, i0:i0 + isz, :]
                xsh_c[s, :isz, ci * NV:(ci + 1) * NV] = xsh[tp, i0:i0 + isz, :]
        in_maps.append({"ker": ker_c, "xsh": xsh_c})

    nc = _build_program()
    res = run_bass_kernel_spmd(nc, in_maps, list(range(NCORES)))
    LAST = res

    out = np.zeros((NCH, NV), np.float32)
    for c in range(NCORES):
        arr = res.results[c]["out_part"]          # (128, 3*512) packed o-chunks
        for oc, (o0, osz) in enumerate(CHUNKS):
            out[o0:o0 + osz] += arr[:osz, oc * NV:(oc + 1) * NV]
    out = out.reshape(1, DIM, Q, P, P, P).copy()
    out[:, :C0] += np.asarray(bias, np.float32).reshape(1, C0, 1, 1, 1, 1)
    return out


# revision 19
# speedup vs baseline: 1849987.4432x; 1849987.4432x over previous
"""Equivariant PQ-layer conv kernel for 8x TRN2 NeuronCores.

Strategy: the layer is a 3D conv (SAME, 5^3 taps) with an assembled
(320, 320, 125) kernel over a (320, 8^3) input. The host assembles the
conv kernel from the geometry/weight inputs (cheap: ~0.2 GFLOP vs the
13 GFLOP conv) and shards the 125 taps across the 8 cores. Each core
gets 8 TAP-PAIR slots (2 taps = 640 contraction rows = exactly 5x128
chunks, so every matmul runs at full K=128). Per pair: 5 contraction
chunks x 3 output chunks of N=512-voxel matmuls (fp16 operands, fp32
PSUM accumulate across all pairs). The host sums the 8 partial outputs
(the unshard for a contraction-parallel split) and adds the bias.
"""
import numpy as np

C0, C1 = 8, 4
K = 5
G = 8
EPS = 1e-6
R_MAX = 5.5
DIM = C0 + 3 * C1          # 20
Q = 16
P = 8
NCH = DIM * Q              # 320
NV = P * P * P             # 512
K3 = K ** 3                # 125
NCORES = 8
PAIRS = 8                  # tap-pair slots per core: 8*8*2 = 128 >= 125 taps
NCC = 5                    # contraction chunks per pair: 2*320/128
OCH = [(0, 128), (128, 128), (256, 64)]

LAST = None                # BassKernelResults of the most recent run (for test harness)
_PROGRAM = None


def _levi_civita():
    e = np.zeros((3, 3, 3), np.float32)
    e[0, 1, 2] = e[1, 2, 0] = e[2, 0, 1] = 1.0
    e[0, 2, 1] = e[2, 1, 0] = e[1, 0, 2] = -1.0
    return e


def _assemble_kern(q_in, q_out, w_ss, w_vs, w_sv, w_vv0, w_vv1):
    """Mirror of the reference kernel assembly, in f32 numpy. -> (320, 320, 125)."""
    offs = np.arange(K, dtype=np.float32) - (K - 1) / 2.0
    oz, oy, ox = np.meshgrid(offs, offs, offs, indexing='ij')
    p_off = np.stack([oz, oy, ox], -1).reshape(-1, 3)
    v = p_off[None, None] - (q_out[:, None, None] - q_in[None, :, None])
    r = np.linalg.norm(v, axis=-1)
    u = np.where(r[..., None] > EPS, v / np.maximum(r, EPS)[..., None], 0.0).astype(np.float32)
    centers = np.linspace(0.0, R_MAX, G).astype(np.float32)
    sigma = R_MAX / (G - 1)
    R = np.exp(-0.5 * ((r[..., None] - centers) / sigma) ** 2).astype(np.float32)
    RY = R[..., None] * u[..., None, :]
    eye3 = np.eye(3, dtype=np.float32)
    eps3 = _levi_civita()
    K_ss = np.einsum('acg,pqkg->apcqk', w_ss, R, optimize=True)
    K_vs = np.einsum('acg,pqkgm->ampcqk', w_vs, RY, optimize=True)
    K_sv = np.einsum('acg,pqkgm->apcmqk', w_sv, RY, optimize=True)
    K_vv = (np.einsum('acg,pqkg,mn->ampcnqk', w_vv0, R, eye3, optimize=True)
            + np.float32(0.7071067811865476) *
            np.einsum('acg,pqkgm,imj->aipcjqk', w_vv1, RY, eps3, optimize=True))
    Qo, Qi = q_out.shape[0], q_in.shape[0]
    top = np.concatenate([K_ss, K_sv.reshape(C0, Qo, 3 * C1, Qi, K3)], axis=2)
    bot = np.concatenate([K_vs.reshape(3 * C1, Qo, C0, Qi, K3),
                          K_vv.reshape(3 * C1, Qo, 3 * C1, Qi, K3)], axis=2)
    kern = np.concatenate([top, bot], axis=0)
    return np.ascontiguousarray(kern.reshape(DIM * Qo, DIM * Qi, K3).astype(np.float32))


def _build_program():
    """One SPMD program: 8 tap-pair slots of (ker, xsh) -> partial conv output.

    Raw bass (no Tile): explicit engine blocks + standalone wait_ge
    instructions, one DMA-completion semaphore per pair slot.
    """
    global _PROGRAM
    if _PROGRAM is not None:
        return _PROGRAM
    from contextlib import ExitStack
    from concourse import bass, mybir

    nc = bass.Bass("TRN2", target_bir_lowering=False, debug=False,
                   enable_asserts=False, num_devices=NCORES)
    # Contraction chunks cc=0..4 packed side-by-side in the free dim.
    ker_d = nc.dram_tensor("ker", [PAIRS, 128, NCC * NCH], mybir.dt.float16,
                           kind="ExternalInput").ap()
    xsh_d = nc.dram_tensor("xsh", [PAIRS, 128, NCC * NV], mybir.dt.float16,
                           kind="ExternalInput").ap()
    # (128, 3*512): o-chunk oc lives in columns [oc*512, (oc+1)*512); host unpacks.
    out_d = nc.dram_tensor("out_part", [128, 3 * NV], mybir.dt.float32,
                           kind="ExternalOutput").ap()

    with ExitStack() as ctx:
        ktiles = [ctx.enter_context(nc.sbuf_tensor(f"kt{p}", [128, NCC * NCH], mybir.dt.float16))
                  for p in range(PAIRS)]
        xtiles = [ctx.enter_context(nc.sbuf_tensor(f"xt{p}", [128, NCC * NV], mybir.dt.float16))
                  for p in range(PAIRS)]
        otile = ctx.enter_context(nc.sbuf_tensor("otile", [128, 3 * NV], mybir.dt.float32))
        psum = [ctx.enter_context(nc.psum_tensor(f"psum{i}", [osz, NV], mybir.dt.float32))
                for i, (o0, osz) in enumerate(OCH)]
        dsem = [ctx.enter_context(nc.semaphore(f"dsem{p}")) for p in range(PAIRS)]
        psem = ctx.enter_context(nc.semaphore("psem"))
        vsem = ctx.enter_context(nc.semaphore("vsem"))
        osem = ctx.enter_context(nc.semaphore("osem"))
        block = ctx.enter_context(nc.Block())

        # ker loads on the SP HWDGE queue, xsh loads on the ACT HWDGE queue:
        # two independent queues cover the ~8.5 MB load stream.
        @block.sync
        def _(sync):
            for p in range(PAIRS):
                sync.dma_start(out=ktiles[p][:, :], in_=ker_d[p, :, :]).then_inc(dsem[p], 16)

        @block.scalar
        def _(scalar):
            for p in range(PAIRS):
                scalar.dma_start(out=xtiles[p][:, :], in_=xsh_d[p, :, :]).then_inc(dsem[p], 16)

        @block.tensor
        def _(tensor):
            for p in range(PAIRS):
                tensor.wait_ge(dsem[p], 32)
                for oc, (o0, osz) in enumerate(OCH):
                    for cc in range(NCC):
                        mm = tensor.matmul(
                            psum[oc][:, :],
                            ktiles[p][:, cc * NCH + o0:cc * NCH + o0 + osz],
                            xtiles[p][:, cc * NV:(cc + 1) * NV],
                            start=(p == 0 and cc == 0),
                            stop=(p == PAIRS - 1 and cc == NCC - 1))
                        if p == PAIRS - 1 and oc == len(OCH) - 1 and cc == NCC - 1:
                            mm.then_inc(psem, 1)

        @block.vector
        def _(vector):
            vector.wait_ge(psem, 1)
            for oc, (o0, osz) in enumerate(OCH):
                vector.tensor_copy(
                    otile[:osz, oc * NV:(oc + 1) * NV], psum[oc][:, :]
                ).then_inc(vsem, 1)

        @block.gpsimd
        def _(gpsimd):
            gpsimd.wait_ge(vsem, len(OCH))
            gpsimd.dma_start(out=out_d[:, :], in_=otile[:, :]).then_inc(osem, 16)
            gpsimd.wait_ge(osem, 16)

    _PROGRAM = nc
    return nc


def kernel(x, q_in, q_out, w_ss, w_vs, w_sv, w_vv0, w_vv1, bias):
    global LAST
    from concourse.bass_utils import run_bass_kernel_spmd

    kern = _assemble_kern(np.asarray(q_in, np.float32), np.asarray(q_out, np.float32),
                          np.asarray(w_ss, np.float32), np.asarray(w_vs, np.float32),
                          np.asarray(w_sv, np.float32), np.asarray(w_vv0, np.float32),
                          np.asarray(w_vv1, np.float32))
    xr = np.asarray(x, np.float32).reshape(NCH, P, P, P)
    x_pad = np.zeros((NCH, P + 4, P + 4, P + 4), np.float32)
    x_pad[:, 2:10, 2:10, 2:10] = xr

    # Shifted input per tap (+1 zero slab for padding slots), fp16.
    xsh = np.zeros((K3 + 1, NCH, NV), np.float16)
    t = 0
    for dz in range(K):
        for dy in range(K):
            for dx in range(K):
                xsh[t] = x_pad[:, dz:dz + 8, dy:dy + 8, dx:dx + 8].reshape(NCH, NV)
                t += 1
    kerT = np.zeros((K3 + 1, NCH, NCH), np.float16)          # (tap, i, o)
    kerT[:K3] = kern.transpose(2, 1, 0)

    in_maps = []
    for c in range(NCORES):
        taps = list(range(c, K3, NCORES)) + [K3] * (2 * PAIRS)  # pad w/ zero slab
        taps = taps[:2 * PAIRS]
        ker_c = np.empty((PAIRS, 128, NCC * NCH), np.float16)
        xsh_c = np.empty((PAIRS, 128, NCC * NV), np.float16)
        for p in range(PAIRS):
            tA, tB = taps[2 * p], taps[2 * p + 1]
            kb = np.concatenate([kerT[tA], kerT[tB]], axis=0)    # (640, 320)
            xb = np.concatenate([xsh[tA], xsh[tB]], axis=0)      # (640, 512)
            ker_c[p] = kb.reshape(NCC, 128, NCH).transpose(1, 0, 2).reshape(128, NCC * NCH)
            xsh_c[p] = xb.reshape(NCC, 128, NV).transpose(1, 0, 2).reshape(128, NCC * NV)
        in_maps.append({"ker": ker_c, "xsh": xsh_c})

    nc = _build_program()
    res = run_bass_kernel_spmd(nc, in_maps, list(range(NCORES)))
    LAST = res

    out = np.zeros((NCH, NV), np.float32)
    for c in range(NCORES):
        arr = res.results[c]["out_part"]          # (128, 3*512) packed o-chunks
        for oc, (o0, osz) in enumerate(OCH):
            out[o0:o0 + osz] += arr[:osz, oc * NV:(oc + 1) * NV]
    out = out.reshape(1, DIM, Q, P, P, P).copy()
    out[:, :C0] += np.asarray(bias, np.float32).reshape(1, C0, 1, 1, 1, 1)
    return out


# revision 20
# speedup vs baseline: 1890041.3486x; 1.0217x over previous
"""Equivariant PQ-layer conv kernel for 8x TRN2 NeuronCores.

Strategy: the layer is a 3D conv (SAME, 5^3 taps) with an assembled
(320, 320, 125) kernel over a (320, 8^3) input. The host assembles the
conv kernel from the geometry/weight inputs (cheap: ~0.2 GFLOP vs the
13 GFLOP conv) and shards the 125 taps across the 8 cores. Each core
gets 8 TAP-PAIR slots (2 taps = 640 contraction rows = exactly 5x128
chunks, so every matmul runs at full K=128). Per pair: 5 contraction
chunks x 3 output chunks of N=512-voxel matmuls (fp16 operands, fp32
PSUM accumulate across all pairs). The host sums the 8 partial outputs
(the unshard for a contraction-parallel split) and adds the bias.
"""
import numpy as np

C0, C1 = 8, 4
K = 5
G = 8
EPS = 1e-6
R_MAX = 5.5
DIM = C0 + 3 * C1          # 20
Q = 16
P = 8
NCH = DIM * Q              # 320
NV = P * P * P             # 512
K3 = K ** 3                # 125
NCORES = 8
PAIRS = 8                  # tap-pair slots per core: 8*8*2 = 128 >= 125 taps
NCC = 5                    # contraction chunks per pair: 2*320/128
OCH = [(0, 128), (128, 128), (256, 64)]

LAST = None                # BassKernelResults of the most recent run (for test harness)
_PROGRAM = None


def _levi_civita():
    e = np.zeros((3, 3, 3), np.float32)
    e[0, 1, 2] = e[1, 2, 0] = e[2, 0, 1] = 1.0
    e[0, 2, 1] = e[2, 1, 0] = e[1, 0, 2] = -1.0
    return e


def _assemble_kern(q_in, q_out, w_ss, w_vs, w_sv, w_vv0, w_vv1):
    """Mirror of the reference kernel assembly, in f32 numpy. -> (320, 320, 125)."""
    offs = np.arange(K, dtype=np.float32) - (K - 1) / 2.0
    oz, oy, ox = np.meshgrid(offs, offs, offs, indexing='ij')
    p_off = np.stack([oz, oy, ox], -1).reshape(-1, 3)
    v = p_off[None, None] - (q_out[:, None, None] - q_in[None, :, None])
    r = np.linalg.norm(v, axis=-1)
    u = np.where(r[..., None] > EPS, v / np.maximum(r, EPS)[..., None], 0.0).astype(np.float32)
    centers = np.linspace(0.0, R_MAX, G).astype(np.float32)
    sigma = R_MAX / (G - 1)
    R = np.exp(-0.5 * ((r[..., None] - centers) / sigma) ** 2).astype(np.float32)
    RY = R[..., None] * u[..., None, :]
    eye3 = np.eye(3, dtype=np.float32)
    eps3 = _levi_civita()
    K_ss = np.einsum('acg,pqkg->apcqk', w_ss, R, optimize=True)
    K_vs = np.einsum('acg,pqkgm->ampcqk', w_vs, RY, optimize=True)
    K_sv = np.einsum('acg,pqkgm->apcmqk', w_sv, RY, optimize=True)
    K_vv = (np.einsum('acg,pqkg,mn->ampcnqk', w_vv0, R, eye3, optimize=True)
            + np.float32(0.7071067811865476) *
            np.einsum('acg,pqkgm,imj->aipcjqk', w_vv1, RY, eps3, optimize=True))
    Qo, Qi = q_out.shape[0], q_in.shape[0]
    top = np.concatenate([K_ss, K_sv.reshape(C0, Qo, 3 * C1, Qi, K3)], axis=2)
    bot = np.concatenate([K_vs.reshape(3 * C1, Qo, C0, Qi, K3),
                          K_vv.reshape(3 * C1, Qo, 3 * C1, Qi, K3)], axis=2)
    kern = np.concatenate([top, bot], axis=0)
    return np.ascontiguousarray(kern.reshape(DIM * Qo, DIM * Qi, K3).astype(np.float32))


def _build_program():
    """One SPMD program: 8 tap-pair slots of (ker, xsh) -> partial conv output.

    Raw bass (no Tile): explicit engine blocks + standalone wait_ge
    instructions, one DMA-completion semaphore per pair slot.
    """
    global _PROGRAM
    if _PROGRAM is not None:
        return _PROGRAM
    from contextlib import ExitStack
    from concourse import bass, mybir

    nc = bass.Bass("TRN2", target_bir_lowering=False, debug=False,
                   enable_asserts=False, num_devices=NCORES)
    # Contraction chunks cc=0..4 packed side-by-side in the free dim.
    ker_d = nc.dram_tensor("ker", [PAIRS, 128, NCC * NCH], mybir.dt.float16,
                           kind="ExternalInput").ap()
    xsh_d = nc.dram_tensor("xsh", [PAIRS, 128, NCC * NV], mybir.dt.float16,
                           kind="ExternalInput").ap()
    # (128, 3*512): o-chunk oc lives in columns [oc*512, (oc+1)*512); host unpacks.
    out_d = nc.dram_tensor("out_part", [128, 3 * NV], mybir.dt.float32,
                           kind="ExternalOutput").ap()

    with ExitStack() as ctx:
        ktiles = [ctx.enter_context(nc.sbuf_tensor(f"kt{p}", [128, NCC * NCH], mybir.dt.float16))
                  for p in range(PAIRS)]
        xtiles = [ctx.enter_context(nc.sbuf_tensor(f"xt{p}", [128, NCC * NV], mybir.dt.float16))
                  for p in range(PAIRS)]
        otile = ctx.enter_context(nc.sbuf_tensor("otile", [128, 3 * NV], mybir.dt.float32))
        psum = [ctx.enter_context(nc.psum_tensor(f"psum{i}", [osz, NV], mybir.dt.float32))
                for i, (o0, osz) in enumerate(OCH)]
        dsem = [ctx.enter_context(nc.semaphore(f"dsem{p}")) for p in range(PAIRS)]
        psem = ctx.enter_context(nc.semaphore("psem"))
        vsem = ctx.enter_context(nc.semaphore("vsem"))
        osem = ctx.enter_context(nc.semaphore("osem"))
        block = ctx.enter_context(nc.Block())

        # ker loads on the SP HWDGE queue, xsh loads on the ACT HWDGE queue:
        # two independent queues cover the ~8.5 MB load stream.
        @block.sync
        def _(sync):
            for p in range(PAIRS):
                sync.dma_start(out=ktiles[p][:, :], in_=ker_d[p, :, :]).then_inc(dsem[p], 16)

        @block.scalar
        def _(scalar):
            for p in range(PAIRS):
                scalar.dma_start(out=xtiles[p][:, :], in_=xsh_d[p, :, :]).then_inc(dsem[p], 16)

        @block.tensor
        def _(tensor):
            for p in range(PAIRS):
                tensor.wait_ge(dsem[p], 32)
                for oc, (o0, osz) in enumerate(OCH):
                    for cc in range(NCC):
                        mm = tensor.matmul(
                            psum[oc][:, :],
                            ktiles[p][:, cc * NCH + o0:cc * NCH + o0 + osz],
                            xtiles[p][:, cc * NV:(cc + 1) * NV],
                            start=(p == 0 and cc == 0),
                            stop=(p == PAIRS - 1 and cc == NCC - 1))
                        if p == PAIRS - 1 and oc == len(OCH) - 1 and cc == NCC - 1:
                            mm.then_inc(psem, 1)

        @block.vector
        def _(vector):
            vector.wait_ge(psem, 1)
            for oc, (o0, osz) in enumerate(OCH):
                vector.tensor_copy(
                    otile[:osz, oc * NV:(oc + 1) * NV], psum[oc][:, :]
                ).then_inc(vsem, 1)

        @block.gpsimd
        def _(gpsimd):
            # store each o-chunk as soon as its PSUM->SBUF copy lands,
            # overlapping the store stream with the remaining copies
            for oc, (o0, osz) in enumerate(OCH):
                gpsimd.wait_ge(vsem, oc + 1)
                gpsimd.dma_start(out=out_d[:osz, oc * NV:(oc + 1) * NV],
                                 in_=otile[:osz, oc * NV:(oc + 1) * NV]).then_inc(osem, 16)
            gpsimd.wait_ge(osem, 16 * len(OCH))

    _PROGRAM = nc
    return nc


def kernel(x, q_in, q_out, w_ss, w_vs, w_sv, w_vv0, w_vv1, bias):
    global LAST
    from concourse.bass_utils import run_bass_kernel_spmd

    kern = _assemble_kern(np.asarray(q_in, np.float32), np.asarray(q_out, np.float32),
                          np.asarray(w_ss, np.float32), np.asarray(w_vs, np.float32),
                          np.asarray(w_sv, np.float32), np.asarray(w_vv0, np.float32),
                          np.asarray(w_vv1, np.float32))
    xr = np.asarray(x, np.float32).reshape(NCH, P, P, P)
    x_pad = np.zeros((NCH, P + 4, P + 4, P + 4), np.float32)
    x_pad[:, 2:10, 2:10, 2:10] = xr

    # Shifted input per tap (+1 zero slab for padding slots), fp16.
    xsh = np.zeros((K3 + 1, NCH, NV), np.float16)
    t = 0
    for dz in range(K):
        for dy in range(K):
            for dx in range(K):
                xsh[t] = x_pad[:, dz:dz + 8, dy:dy + 8, dx:dx + 8].reshape(NCH, NV)
                t += 1
    kerT = np.zeros((K3 + 1, NCH, NCH), np.float16)          # (tap, i, o)
    kerT[:K3] = kern.transpose(2, 1, 0)

    in_maps = []
    for c in range(NCORES):
        taps = list(range(c, K3, NCORES)) + [K3] * (2 * PAIRS)  # pad w/ zero slab
        taps = taps[:2 * PAIRS]
        ker_c = np.empty((PAIRS, 128, NCC * NCH), np.float16)
        xsh_c = np.empty((PAIRS, 128, NCC * NV), np.float16)
        for p in range(PAIRS):
            tA, tB = taps[2 * p], taps[2 * p + 1]
            kb = np.concatenate([kerT[tA], kerT[tB]], axis=0)    # (640, 320)
            xb = np.concatenate([xsh[tA], xsh[tB]], axis=0)      # (640, 512)
            ker_c[p] = kb.reshape(NCC, 128, NCH).transpose(1, 0, 2).reshape(128, NCC * NCH)
            xsh_c[p] = xb.reshape(NCC, 128, NV).transpose(1, 0, 2).reshape(128, NCC * NV)
        in_maps.append({"ker": ker_c, "xsh": xsh_c})

    nc = _build_program()
    res = run_bass_kernel_spmd(nc, in_maps, list(range(NCORES)))
    LAST = res

    out = np.zeros((NCH, NV), np.float32)
    for c in range(NCORES):
        arr = res.results[c]["out_part"]          # (128, 3*512) packed o-chunks
        for oc, (o0, osz) in enumerate(OCH):
            out[o0:o0 + osz] += arr[:osz, oc * NV:(oc + 1) * NV]
    out = out.reshape(1, DIM, Q, P, P, P).copy()
    out[:, :C0] += np.asarray(bias, np.float32).reshape(1, C0, 1, 1, 1, 1)
    return out


# revision 21
# speedup vs baseline: 2179285.3787x; 1.1530x over previous
"""Equivariant PQ-layer conv kernel for 8x TRN2 NeuronCores.

Strategy: the layer is a 3D conv (SAME, 5^3 taps) with an assembled
(320, 320, 125) kernel over a (320, 8^3) input. The host assembles the
conv kernel from the geometry/weight inputs (cheap: ~0.2 GFLOP vs the
13 GFLOP conv) and shards the 125 taps across the 8 cores. Each core
gets 8 TAP-PAIR slots (2 taps = 640 contraction rows = exactly 5x128
chunks, so every matmul runs at full K=128). Per pair: 5 contraction
chunks x 3 output chunks of N=512-voxel matmuls (fp16 operands, fp32
PSUM accumulate across all pairs). The host sums the 8 partial outputs
(the unshard for a contraction-parallel split) and adds the bias.
"""
import numpy as np

C0, C1 = 8, 4
K = 5
G = 8
EPS = 1e-6
R_MAX = 5.5
DIM = C0 + 3 * C1          # 20
Q = 16
P = 8
NCH = DIM * Q              # 320
NV = P * P * P             # 512
K3 = K ** 3                # 125
NCORES = 8
PAIRS = 8                  # tap-pair slots per core: 8*8*2 = 128 >= 125 taps
NCC = 5                    # contraction chunks per pair: 2*320/128
OCH = [(0, 128), (128, 128), (256, 64)]

LAST = None                # BassKernelResults of the most recent run (for test harness)
_PROGRAM = None


def _levi_civita():
    e = np.zeros((3, 3, 3), np.float32)
    e[0, 1, 2] = e[1, 2, 0] = e[2, 0, 1] = 1.0
    e[0, 2, 1] = e[2, 1, 0] = e[1, 0, 2] = -1.0
    return e


def _assemble_kern(q_in, q_out, w_ss, w_vs, w_sv, w_vv0, w_vv1):
    """Mirror of the reference kernel assembly, in f32 numpy. -> (320, 320, 125)."""
    offs = np.arange(K, dtype=np.float32) - (K - 1) / 2.0
    oz, oy, ox = np.meshgrid(offs, offs, offs, indexing='ij')
    p_off = np.stack([oz, oy, ox], -1).reshape(-1, 3)
    v = p_off[None, None] - (q_out[:, None, None] - q_in[None, :, None])
    r = np.linalg.norm(v, axis=-1)
    u = np.where(r[..., None] > EPS, v / np.maximum(r, EPS)[..., None], 0.0).astype(np.float32)
    centers = np.linspace(0.0, R_MAX, G).astype(np.float32)
    sigma = R_MAX / (G - 1)
    R = np.exp(-0.5 * ((r[..., None] - centers) / sigma) ** 2).astype(np.float32)
    RY = R[..., None] * u[..., None, :]
    eye3 = np.eye(3, dtype=np.float32)
    eps3 = _levi_civita()
    K_ss = np.einsum('acg,pqkg->apcqk', w_ss, R, optimize=True)
    K_vs = np.einsum('acg,pqkgm->ampcqk', w_vs, RY, optimize=True)
    K_sv = np.einsum('acg,pqkgm->apcmqk', w_sv, RY, optimize=True)
    K_vv = (np.einsum('acg,pqkg,mn->ampcnqk', w_vv0, R, eye3, optimize=True)
            + np.float32(0.7071067811865476) *
            np.einsum('acg,pqkgm,imj->aipcjqk', w_vv1, RY, eps3, optimize=True))
    Qo, Qi = q_out.shape[0], q_in.shape[0]
    top = np.concatenate([K_ss, K_sv.reshape(C0, Qo, 3 * C1, Qi, K3)], axis=2)
    bot = np.concatenate([K_vs.reshape(3 * C1, Qo, C0, Qi, K3),
                          K_vv.reshape(3 * C1, Qo, 3 * C1, Qi, K3)], axis=2)
    kern = np.concatenate([top, bot], axis=0)
    return np.ascontiguousarray(kern.reshape(DIM * Qo, DIM * Qi, K3).astype(np.float32))


def _build_program():
    """One SPMD program: 8 tap-pair slots of (ker, xsh) -> partial conv output.

    Raw bass (no Tile): explicit engine blocks + standalone wait_ge
    instructions, one DMA-completion semaphore per pair slot.
    """
    global _PROGRAM
    if _PROGRAM is not None:
        return _PROGRAM
    from contextlib import ExitStack
    from concourse import bass, mybir

    nc = bass.Bass("TRN2", target_bir_lowering=False, debug=False,
                   enable_asserts=False, num_devices=NCORES)
    # Contraction chunks cc=0..4 packed side-by-side in the free dim.
    ker_d = nc.dram_tensor("ker", [PAIRS, 128, NCC * NCH], mybir.dt.float16,
                           kind="ExternalInput").ap()
    xsh_d = nc.dram_tensor("xsh", [PAIRS, 128, NCC * NV], mybir.dt.float16,
                           kind="ExternalInput").ap()
    # (128, 3*512): o-chunk oc lives in columns [oc*512, (oc+1)*512); host unpacks.
    out_d = nc.dram_tensor("out_part", [128, 3 * NV], mybir.dt.float32,
                           kind="ExternalOutput").ap()

    with ExitStack() as ctx:
        ktiles = [ctx.enter_context(nc.sbuf_tensor(f"kt{p}", [128, NCC * NCH], mybir.dt.float16))
                  for p in range(PAIRS)]
        xtiles = [ctx.enter_context(nc.sbuf_tensor(f"xt{p}", [128, NCC * NV], mybir.dt.float16))
                  for p in range(PAIRS)]
        otile = ctx.enter_context(nc.sbuf_tensor("otile", [128, 3 * NV], mybir.dt.float32))
        psum = [ctx.enter_context(nc.psum_tensor(f"psum{i}", [osz, NV], mybir.dt.float32))
                for i, (o0, osz) in enumerate(OCH)]
        dsemA = [ctx.enter_context(nc.semaphore(f"dsemA{p}")) for p in range(PAIRS)]
        dsemB = [ctx.enter_context(nc.semaphore(f"dsemB{p}")) for p in range(PAIRS)]
        psem = ctx.enter_context(nc.semaphore("psem"))
        vsem = ctx.enter_context(nc.semaphore("vsem"))
        osem = ctx.enter_context(nc.semaphore("osem"))
        block = ctx.enter_context(nc.Block())

        # ker loads on the SP HWDGE queue, xsh loads on the ACT HWDGE queue;
        # each pair's load is split in two halves (chunks 0-1 / 2-4) with
        # per-half sems so the PE starts on a pair after ~40% of its bytes.
        nA_k, nA_x = 2 * NCH, 2 * NV
        HALVES = (([0, 1], dsemA), ([2, 3, 4], dsemB))

        @block.sync
        def _(sync):
            for p in range(PAIRS):
                sync.dma_start(out=ktiles[p][:, :nA_k], in_=ker_d[p, :, :nA_k]).then_inc(dsemA[p], 16)
                sync.dma_start(out=ktiles[p][:, nA_k:], in_=ker_d[p, :, nA_k:]).then_inc(dsemB[p], 16)

        @block.scalar
        def _(scalar):
            for p in range(PAIRS):
                scalar.dma_start(out=xtiles[p][:, :nA_x], in_=xsh_d[p, :, :nA_x]).then_inc(dsemA[p], 16)
                scalar.dma_start(out=xtiles[p][:, nA_x:], in_=xsh_d[p, :, nA_x:]).then_inc(dsemB[p], 16)

        @block.tensor
        def _(tensor):
            for p in range(PAIRS):
                for ccs, sems in HALVES:
                    tensor.wait_ge(sems[p], 32)
                    for oc, (o0, osz) in enumerate(OCH):
                        for cc in ccs:
                            mm = tensor.matmul(
                                psum[oc][:, :],
                                ktiles[p][:, cc * NCH + o0:cc * NCH + o0 + osz],
                                xtiles[p][:, cc * NV:(cc + 1) * NV],
                                start=(p == 0 and cc == 0),
                                stop=(p == PAIRS - 1 and cc == NCC - 1))
                            if p == PAIRS - 1 and oc == len(OCH) - 1 and cc == NCC - 1:
                                mm.then_inc(psem, 1)

        @block.vector
        def _(vector):
            vector.wait_ge(psem, 1)
            for oc, (o0, osz) in enumerate(OCH):
                vector.tensor_copy(
                    otile[:osz, oc * NV:(oc + 1) * NV], psum[oc][:, :]
                ).then_inc(vsem, 1)

        @block.gpsimd
        def _(gpsimd):
            # store each o-chunk as soon as its PSUM->SBUF copy lands,
            # overlapping the store stream with the remaining copies
            for oc, (o0, osz) in enumerate(OCH):
                gpsimd.wait_ge(vsem, oc + 1)
                gpsimd.dma_start(out=out_d[:osz, oc * NV:(oc + 1) * NV],
                                 in_=otile[:osz, oc * NV:(oc + 1) * NV]).then_inc(osem, 16)
            gpsimd.wait_ge(osem, 16 * len(OCH))

    _PROGRAM = nc
    return nc


def kernel(x, q_in, q_out, w_ss, w_vs, w_sv, w_vv0, w_vv1, bias):
    global LAST
    from concourse.bass_utils import run_bass_kernel_spmd

    kern = _assemble_kern(np.asarray(q_in, np.float32), np.asarray(q_out, np.float32),
                          np.asarray(w_ss, np.float32), np.asarray(w_vs, np.float32),
                          np.asarray(w_sv, np.float32), np.asarray(w_vv0, np.float32),
                          np.asarray(w_vv1, np.float32))
    xr = np.asarray(x, np.float32).reshape(NCH, P, P, P)
    x_pad = np.zeros((NCH, P + 4, P + 4, P + 4), np.float32)
    x_pad[:, 2:10, 2:10, 2:10] = xr

    # Shifted input per tap (+1 zero slab for padding slots), fp16.
    xsh = np.zeros((K3 + 1, NCH, NV), np.float16)
    t = 0
    for dz in range(K):
        for dy in range(K):
            for dx in range(K):
                xsh[t] = x_pad[:, dz:dz + 8, dy:dy + 8, dx:dx + 8].reshape(NCH, NV)
                t += 1
    kerT = np.zeros((K3 + 1, NCH, NCH), np.float16)          # (tap, i, o)
    kerT[:K3] = kern.transpose(2, 1, 0)

    in_maps = []
    for c in range(NCORES):
        taps = list(range(c, K3, NCORES)) + [K3] * (2 * PAIRS)  # pad w/ zero slab
        taps = taps[:2 * PAIRS]
        ker_c = np.empty((PAIRS, 128, NCC * NCH), np.float16)
        xsh_c = np.empty((PAIRS, 128, NCC * NV), np.float16)
        for p in range(PAIRS):
            tA, tB = taps[2 * p], taps[2 * p + 1]
            kb = np.concatenate([kerT[tA], kerT[tB]], axis=0)    # (640, 320)
            xb = np.concatenate([xsh[tA], xsh[tB]], axis=0)      # (640, 512)
            ker_c[p] = kb.reshape(NCC, 128, NCH).transpose(1, 0, 2).reshape(128, NCC * NCH)
            xsh_c[p] = xb.reshape(NCC, 128, NV).transpose(1, 0, 2).reshape(128, NCC * NV)
        in_maps.append({"ker": ker_c, "xsh": xsh_c})

    nc = _build_program()
    res = run_bass_kernel_spmd(nc, in_maps, list(range(NCORES)))
    LAST = res

    out = np.zeros((NCH, NV), np.float32)
    for c in range(NCORES):
        arr = res.results[c]["out_part"]          # (128, 3*512) packed o-chunks
        for oc, (o0, osz) in enumerate(OCH):
            out[o0:o0 + osz] += arr[:osz, oc * NV:(oc + 1) * NV]
    out = out.reshape(1, DIM, Q, P, P, P).copy()
    out[:, :C0] += np.asarray(bias, np.float32).reshape(1, C0, 1, 1, 1, 1)
    return out
